# revision 1
# baseline (speedup 1.0000x reference)
"""Trainium2 Bass kernel: 16-head causal attention (B=4, S=2048, E=1024).

Sharding: 8 cores = 4 batches x 2 head-groups (8 heads each); host sums the
two head-group partials (fp32) and adds bo.

Per-core pipeline (fp16/bf16 matmul operands; PSUM accumulates fp32):
  - q^T = Wq_g X^T, k^T = Wk_g X^T    (transposed projections, [dq, S] f16)
  - V   = X^T.T Wv_g^T                (natural [S, dv] bf16, +ones column per
                                       head so PV also yields denominators)
  - scores^T[k, q] at 128x128 causal granularity: fully-masked sub-blocks are
    skipped; each diagonal-crossing sub-block gets one [128,128] additive mask
    matmul (identity stationary, f16 mask moving, NEG=-60000).
  - P^T = exp(scores^T/8) on ACT -> bf16 (range-safe: exp can reach ~1.3e8,
    which overflows f16; masked lanes underflow to exactly 0)
  - PV: out[q, 65] += P^T_block^T V_aug: stationary = P^T [128,128], moving =
    V_aug [128,65] bf16 -> full 128 output partitions at 65 rows/block. One
    PSUM accumulation group per vpa bank (single start/stop; sub-regions
    auto-initialize via the pending-zero mechanism).
  - normalize: DVE reciprocal of the denominator column + tensor_scalar_mul
  - attn [q, dq] f16 -> PE-transpose [dq, q] -> Wo matmul -> f16 partials
Scheduling: the emitter interleaves projection/output-projection work into the
ACT-bound attention windows (deadline queue + PE-vs-ACT balance heuristic),
batches DMAs into ~45 large transfers, and software-pipelines scores/exp/PV
with a lag of one exp group.
"""

import contextlib

import numpy as np

import bass_rust
import concourse.bass as bass
import concourse.mybir as mybir
import concourse.tile as tile

F32 = mybir.dt.float32
F16 = mybir.dt.float16
BF16 = mybir.dt.bfloat16
AF = mybir.ActivationFunctionType

B, S, E = 4, 2048, 1024
H, D = 16, 64
NCORES = 8
NGROUPS = 2            # head groups (tensor parallel)
HPC = H // NGROUPS     # heads per core
DQ = HPC * D           # per-core projection width = 512
NEG = -60000.0         # f16-representable; exp(NEG/8) == 0.0 in fp32

SK = 128               # k sub-block (partition dim of scores^T)
SQ = 512               # q window
GW = 1024              # exp group width (psum [128, GW])


def split_excess_waits(nc, maxw=1):
    """This container's walrus supports one sem wait per instruction;
    hoist extras onto same-engine nops just before the instruction."""
    n_new = 0
    for bb in nc.main_func.blocks:
        new_list = []
        changed = False
        for inst in list(bb.instructions):
            si = inst.sync_info
            waits = list(si.on_wait) if si and si.on_wait else []
            if len(waits) > maxw:
                changed = True
                extra, keep = waits[:-maxw], waits[-maxw:]
                for ci in range(0, len(extra), maxw):
                    nop = bass_rust.InstNoOp(
                        name=f"I-waitsplit-{n_new}", ins=[], outs=[]
                    )
                    n_new += 1
                    nop.engine = inst.engine
                    nop.sync_info = mybir.SyncInfo(
                        on_wait=extra[ci : ci + maxw], on_update=[]
                    )
                    new_list.append(nop)
                inst.sync_info = mybir.SyncInfo(
                    on_wait=keep,
                    on_update=list(si.on_update) if si.on_update else [],
                )
            new_list.append(inst)
        if changed:
            bb.instructions = new_list
    return n_new


def build_kernel(causal=True, split_waits=True, debug=False):
    s, e, hpc, d = S, E, HPC, D
    dq = hpc * d              # 512
    nec = e // 128            # 8 input-feature chunks
    ndq = dq // 128           # 4 projection partition chunks
    nwin = s // SQ            # 4 q windows
    nsc = s // 128            # 16 s chunks

    nc = bass.Bass()

    xq = nc.declare_dram_parameter("xq_t", [e, s], F16, isOutput=False)
    xk = nc.declare_dram_parameter("xk_t", [e, s], F16, isOutput=False)
    xv = nc.declare_dram_parameter("xv_t", [e, s], F16, isOutput=False)
    wqd = nc.declare_dram_parameter("wq_t", [e, dq], F16, isOutput=False)
    wkd = nc.declare_dram_parameter("wk_t", [e, dq], F16, isOutput=False)
    wvd = nc.declare_dram_parameter("wv_t", [e, dq], F16, isOutput=False)
    wod = nc.declare_dram_parameter("wo_t", [dq, e], F16, isOutput=False)
    # packed constants: [bq(4) | bk(4) | bv_b(512)] f32, [ident | crossmask] f16
    cfd = nc.declare_dram_parameter("consts_f32", [128, 2 * ndq + dq], F32,
                                    isOutput=False)
    chd = nc.declare_dram_parameter("consts_f16", [128, 256], F16,
                                    isOutput=False)
    out = nc.declare_dram_parameter("out", [s, e], F16, isOutput=True)
    if debug:
        dbg_q = nc.declare_dram_parameter("dbg_q", [dq, s], F16, isOutput=True)
        dbg_k = nc.declare_dram_parameter("dbg_k", [dq, s], F16, isOutput=True)
        dbg_v = nc.declare_dram_parameter(
            "dbg_v", [s, hpc * (d + 1)], BF16, isOutput=True
        )
        dbg_at = nc.declare_dram_parameter("dbg_at", [s, dq], F16, isOutput=True)
        dbg_pt = nc.declare_dram_parameter("dbg_pt", [128, 17408], BF16,
                                           isOutput=True)
        dbg_rc = nc.declare_dram_parameter("dbg_rc", [128, 16], F32,
                                           isOutput=True)
        dbg_off = [0]

    with tile.TileContext(nc) as tc, contextlib.ExitStack() as ctx:
        pers = ctx.enter_context(tc.tile_pool(name="pers", bufs=1))
        xpool = ctx.enter_context(tc.tile_pool(name="xp", bufs=3))
        ppool = ctx.enter_context(tc.tile_pool(name="ppl", bufs=4))
        atn = ctx.enter_context(tc.tile_pool(name="atn", bufs=4))
        att = ctx.enter_context(tc.tile_pool(name="att", bufs=4))
        nrm = ctx.enter_context(tc.tile_pool(name="nrm", bufs=4))
        opool = ctx.enter_context(tc.tile_pool(name="opl", bufs=3))
        pp = ctx.enter_context(tc.tile_pool(name="pp", bufs=2, space="PSUM"))
        sp = ctx.enter_context(tc.tile_pool(name="sp", bufs=2, space="PSUM"))
        vp = ctx.enter_context(tc.tile_pool(name="vp", bufs=2, space="PSUM"))

        # ---- persistent tensors ----
        cf_sb = pers.tile([128, 2 * ndq + dq], F32, name="cf_sb")
        ch_sb = pers.tile([128, 256], F16, name="ch_sb")
        bq_sb = cf_sb[:, 0:ndq]
        bk_sb = cf_sb[:, ndq : 2 * ndq]
        bv_sb = cf_sb[:, 2 * ndq : 2 * ndq + dq]
        id_sb = ch_sb[:, 0:128]
        mk_sb = ch_sb[:, 128:256]
        q_sb = [
            [pers.tile([128, SQ], F16, name=f"q_sb{c}_{w}") for w in range(nwin)]
            for c in range(ndq)
        ]
        k_sb = [
            [pers.tile([128, SQ], F16, name=f"k_sb{c}_{w}") for w in range(nwin)]
            for c in range(ndq)
        ]
        v_sb = [
            pers.tile([128, hpc * (d + 1)], BF16, name=f"v_sb{i}")
            for i in range(nsc)
        ]
        wq_sb = pers.tile([128, nec * dq], F16, name="wq_sb")
        wk_sb = pers.tile([128, nec * dq], F16, name="wk_sb")
        wv_sb = pers.tile([128, nec * dq], F16, name="wv_sb")
        wo_sb = pers.tile([128, ndq * e], F16, name="wo_sb")

        # ---- DMA helpers (SP engine -> one HWDGE queue, program order) ----
        def load_w_part(wt, dst, part, nparts=2):
            # e-chunk group `part` of [e, dq] -> dst cols
            g = nec // nparts
            src = wt.rearrange("(n p) m -> p n m", p=128)
            nc.sync.dma_start(
                out=dst.rearrange("p (n m) -> p n m", m=dq)[
                    :, part * g : (part + 1) * g, :
                ],
                in_=src[:, part * g : (part + 1) * g, :],
            )

        def load_x_slab(xt, dst, sb, part=None, nparts=2):
            # dst: [128, nec*512] tile; cols [sb*512,(sb+1)*512) of [e, s]
            src = xt.rearrange("(n p) m -> p n m", p=128)
            d3 = dst.rearrange("p (n m) -> p n m", m=SQ)
            if part is None:
                nc.sync.dma_start(
                    out=d3[:, :, :],
                    in_=src[:, :, sb * SQ : (sb + 1) * SQ],
                )
            else:
                g = nec // nparts
                nc.sync.dma_start(
                    out=d3[:, part * g : (part + 1) * g, :],
                    in_=src[:, part * g : (part + 1) * g,
                            sb * SQ : (sb + 1) * SQ],
                )



        x_t = {}  # (tensor, sb) -> slab tile
        for t, xd in (("q", xq), ("k", xk), ("v", xv)):
            x_t[t, 0] = xpool.tile([128, nec * SQ], F16, tag=f"x{t}",
                                   name=f"x{t}0", bufs=3)
        # slab 0 interleaved with weight pieces for earliest unblock;
        # wq/xq0 in quarters so the first projection matmuls start ASAP
        for part in range(4):
            load_w_part(wqd, wq_sb, part, nparts=4)
            load_x_slab(xq, x_t["q", 0], 0, part=part, nparts=4)
        # packed constants (biases for the first bias-add, mask for h0 scores)
        nc.sync.dma_start(out=cf_sb[:, :], in_=cfd[:, :])
        nc.sync.dma_start(out=ch_sb[:, :], in_=chd[:, :])
        load_w_part(wkd, wk_sb, 0)
        load_x_slab(xk, x_t["k", 0], 0, part=0)
        load_w_part(wkd, wk_sb, 1)
        load_x_slab(xk, x_t["k", 0], 0, part=1)
        load_w_part(wvd, wv_sb, 0)
        load_x_slab(xv, x_t["v", 0], 0, part=0)
        load_w_part(wvd, wv_sb, 1)
        load_x_slab(xv, x_t["v", 0], 0, part=1)
        x_t["q", 1] = xpool.tile([128, nec * SQ], F16, tag="xq",
                                 name="xq1", bufs=3)
        load_x_slab(xq, x_t["q", 1], 1)
        for sb in range(1, nwin):
            for t, xd in (("q", xq), ("k", xk), ("v", xv)):
                if (t, sb) in x_t:
                    continue
                x_t[t, sb] = xpool.tile([128, nec * SQ], F16, tag=f"x{t}",
                                        name=f"x{t}{sb}", bufs=3)
                load_x_slab(xd, x_t[t, sb], sb)
            if sb == 1:
                nc.sync.dma_start(
                    out=wo_sb.rearrange("p (n m) -> p n m", m=e),
                    in_=wod.rearrange("(n p) m -> p n m", p=128),
                )

        # ones columns of v_sb, once, on the idle gpsimd engine
        for i in range(nsc):
            v3 = v_sb[i].rearrange("p (h t) -> p h t", t=d + 1)
            nc.gpsimd.memset(v3[:, :, d], 1.0)

        # ---- compute unit generators ----
        def w3(wt):
            return wt.rearrange("p (n m) -> p n m", m=dq)

        open_ps = {}

        def proj_qk_phase(w_sb_t, xt, dst, bias, sb, c, phase):
            """Half-contraction phase of a q/k projection unit. Phase 0
            allocates the psum tile and contracts ec 0..3; phase 1 finishes
            ec 4..7 and applies the bias. Between a unit's phases at most one
            other pp allocation may occur (pp bufs=2)."""
            key = ("qk", xt, sb, c)
            if phase == 0:
                ps = pp.tile([128, SQ], F32, tag="pp", name="ps_pj")
                open_ps[key] = ps
                ecs = range(0, nec // 2)
            else:
                ps = open_ps.pop(key)
                ecs = range(nec // 2, nec)
            for ec in ecs:
                nc.tensor.matmul(
                    ps[:, :],
                    w3(w_sb_t)[:, ec, c * 128 : (c + 1) * 128],
                    x_t[xt, sb][:, ec * SQ : (ec + 1) * SQ],
                    start=(ec == 0),
                    stop=(ec == nec - 1),
                )
            pe_rows(nec * SQ // 2)
            if phase == 1:
                nc.vector.tensor_scalar_add(
                    dst[c][sb][:, :], ps[:, :], bias[:, c : c + 1]
                )

        def proj_v_phase(sb, ii, phase):
            key = ("v", sb, ii)
            if phase == 0:
                ps = pp.tile([128, dq], F32, tag="pp", name="ps_v")
                open_ps[key] = ps
                ecs = range(0, nec // 2)
            else:
                ps = open_ps.pop(key)
                ecs = range(nec // 2, nec)
            wv_ = w3(wv_sb)
            for ec in ecs:
                nc.tensor.matmul(
                    ps[:, :],
                    x_t["v", sb][:, ec * SQ + ii * 128 : ec * SQ + ii * 128 + 128],
                    wv_[:, ec, :],
                    start=(ec == 0),
                    stop=(ec == nec - 1),
                )
            pe_rows(nec * SQ // 2)
            if phase == 1:
                i = sb * 4 + ii
                v3 = v_sb[i].rearrange("p (h t) -> p h t", t=d + 1)
                nc.vector.tensor_add(
                    v3[:, :, 0:d],
                    ps[:, :].rearrange("p (h t) -> p h t", t=d),
                    bv_sb[:, :].rearrange("p (h t) -> p h t", t=d),
                )

        def proj_qk_unit(w_sb_t, xt, dst, bias, sb, c):
            """One [128,512] slab-column of a transposed projection."""
            ps = pp.tile([128, SQ], F32, tag="pp", name="ps_pj")
            wv_ = w3(w_sb_t)
            for ec in range(nec):
                nc.tensor.matmul(
                    ps[:, :],
                    wv_[:, ec, c * 128 : (c + 1) * 128],
                    x_t[xt, sb][:, ec * SQ : (ec + 1) * SQ],
                    start=(ec == 0),
                    stop=(ec == nec - 1),
                )
            nc.vector.tensor_scalar_add(
                dst[c][sb][:, :], ps[:, :], bias[:, c : c + 1]
            )

        def proj_v_unit(sb, ii):
            """One [128(s), dq] natural-layout V chunk (i = sb*4+ii)."""
            i = sb * 4 + ii
            ps = pp.tile([128, dq], F32, tag="pp", name="ps_v")
            wv_ = w3(wv_sb)
            for ec in range(nec):
                nc.tensor.matmul(
                    ps[:, :],
                    x_t["v", sb][:, ec * SQ + ii * 128 : ec * SQ + ii * 128 + 128],
                    wv_[:, ec, :],
                    start=(ec == 0),
                    stop=(ec == nec - 1),
                )
            v3 = v_sb[i].rearrange("p (h t) -> p h t", t=d + 1)
            nc.vector.tensor_add(
                v3[:, :, 0:d],
                ps[:, :].rearrange("p (h t) -> p h t", t=d),
                bv_sb[:, :].rearrange("p (h t) -> p h t", t=d),
            )

        # static PE/ACT occupancy estimate driving filler insertion
        eng_ns = {"pe": 0.0, "act": 0.0}

        def pe_rows(n):
            eng_ns["pe"] += n * 0.4167

        def act_cols(n):
            eng_ns["act"] += 1.326 * (n * 0.8333 + 185.0)  # tuned filler bias

        def attention_head(qb, h, att_tiles, pre_last_cb=None,
                           act_norm=False):
            """scores+exp+PV+normalize for one (window, head).

            Generator: yields after each score-group / PV emission so the
            driver can interleave PE filler while ACT churns through exps.
            pre_last_cb: emitted right after the last score group (tail
            shortening for the final head). act_norm: do half the normalize
            multiplies on ACT (only sensible when ACT is idle afterwards).
            """
            c, hp = h // 2, (h % 2) * 64
            nkb = 4 * qb + 4 if causal else nsc
            # segments: (kb, qstart_global, width)
            segs = []
            for kb in range(nkb):
                if causal and kb >= 4 * qb:
                    qs = kb * 128
                else:
                    qs = qb * SQ
                segs.append((kb, qs, (qb + 1) * SQ - qs))
            # greedy-pack into exp groups of width <= GW
            groups, cur, curw = [], [], 0
            for seg in segs:
                if curw + seg[2] > GW:
                    groups.append(cur)
                    cur, curw = [], 0
                cur.append(seg)
                curw += seg[2]
            if cur:
                groups.append(cur)
            if len(groups) > 1:
                # smallest group first: its short exp lands while ACT still
                # drains the previous head, instead of bubbling at head end
                groups = groups[-2:] + groups[:-2]

            vpa = vp.tile([128, 4 * (d + 1)], F32, tag="vo", name="vpa")
            last_kb = nkb - 1
            npv = sum(
                1 for kb in range(nkb) for qcl in range(4)
                if not (causal and 4 * qb + qcl < kb))
            pv_n = [0]

            def emit_scores(grp):
                gw = sum(g[2] for g in grp)
                scp = sp.tile([128, GW], F32, tag="sc", name="scp")
                off = 0
                for kb, qs, w in grp:
                    ks = k_sb[c][kb // 4][hp : hp + d,
                                          (kb % 4) * 128 : (kb % 4) * 128 + 128]
                    qw_ = q_sb[c][qs // SQ]
                    if causal and kb >= 4 * qb:
                        # additive mask for the diagonal-crossing sub-block
                        nc.tensor.matmul(scp[:, off : off + 128], id_sb[:, :],
                                         mk_sb[:, :], start=True, stop=False)
                        nc.tensor.matmul(
                            scp[:, off : off + 128], ks,
                            qw_[hp : hp + d, qs % SQ : qs % SQ + 128],
                            start=False, stop=True,
                        )
                        pe_rows(256)
                        if w > 128:
                            nc.tensor.matmul(
                                scp[:, off + 128 : off + w], ks,
                                qw_[hp : hp + d, qs % SQ + 128 : qs % SQ + w],
                                start=True, stop=True,
                            )
                            pe_rows(w - 128)
                    else:
                        nc.tensor.matmul(
                            scp[:, off : off + w], ks,
                            qw_[hp : hp + d, qs % SQ : qs % SQ + w],
                            start=True, stop=True,
                        )
                        pe_rows(w)
                    off += w
                pt = ppool.tile([128, GW], BF16, tag="pt", name="pt")
                nc.scalar.activation(
                    pt[:, 0:gw], scp[:, 0:gw], AF.Exp,
                    scale=float(1.0 / np.sqrt(d)),
                )
                act_cols(gw)
                if debug and h == 0:
                    nc.sync.dma_start(
                        out=dbg_pt[:, dbg_off[0] : dbg_off[0] + gw],
                        in_=pt[:, 0:gw])
                    dbg_off[0] += gw
                return pt

            def emit_pv(grp, pt):
                # One psum accumulation group for the whole vpa bank: a
                # start marks the full 2KB zero-region pending-zero, so only
                # the first matmul may carry start and only the last stop;
                # each sub-region auto-initializes on its first write.
                off = 0
                for kb, qs, w in grp:
                    for qcl in range(4):
                        qg = 4 * qb + qcl           # global q chunk
                        if causal and qg < kb:
                            continue                 # fully masked block
                        boff = off + qcl * 128 + qb * SQ - qs
                        nc.tensor.matmul(
                            vpa[:, qcl * (d + 1) : (qcl + 1) * (d + 1)],
                            pt[:, boff : boff + 128],
                            v_sb[kb][:, h * (d + 1) : (h + 1) * (d + 1)],
                            start=(pv_n[0] == 0),
                            stop=(pv_n[0] == npv - 1),
                        )
                        pv_n[0] += 1
                        pe_rows(d + 1)
                    off += w

            # lag-1 software pipeline: scores g+1 overlaps exp g
            prev = None
            for gi, grp in enumerate(groups):
                pt = emit_scores(grp)
                if pre_last_cb is not None and gi == len(groups) - 1:
                    pre_last_cb()
                yield
                if prev is not None:
                    emit_pv(*prev)
                    yield
                prev = (grp, pt)
            emit_pv(*prev)

            v4 = vpa.rearrange("p (qc t) -> p qc t", t=d + 1)
            rcp = nrm.tile([128, 4], F32, tag="rcp", name="rcp")
            nc.vector.reciprocal(rcp[:, :], v4[:, :, d])
            if debug and h == 0:
                nc.sync.dma_start(out=dbg_rc[:, qb * 4 : qb * 4 + 4],
                                  in_=rcp[:, :])
            for qcl in range(4):
                if act_norm and qcl >= 2:
                    nc.scalar.activation(
                        att_tiles[qcl][:, h * d : (h + 1) * d],
                        v4[:, qcl, 0:d],
                        AF.Copy,
                        scale=rcp[:, qcl : qcl + 1],
                    )
                else:
                    nc.vector.tensor_scalar_mul(
                        att_tiles[qcl][:, h * d : (h + 1) * d],
                        v4[:, qcl, 0:d],
                        rcp[:, qcl : qcl + 1],
                    )

        def wo_transpose_unit(att_tiles, cc, at_store, copy_eng=None):
            """Transpose attn chunk cc (heads 2cc, 2cc+1) -> at_store[cc]."""
            tp = pp.tile([128, SQ], F16, tag="pp", name="tp")
            for qcl in range(4):
                nc.tensor.transpose(
                    tp[:, qcl * 128 : (qcl + 1) * 128],
                    att_tiles[qcl][:, cc * 128 : (cc + 1) * 128],
                    id_sb[:, :],
                )
                pe_rows(128)
            at_ = att.tile([128, SQ], F16, tag=f"at{cc}", name="at_")
            if copy_eng is None:
                nc.vector.tensor_copy(at_[:, :], tp[:, :])
            else:
                copy_eng.copy(at_[:, :], tp[:, :])
            at_store[cc] = at_

        def wo_matmul_unit(at_store, qb, i, copy_eng=None):
            """Output projection + store for s-chunk i of window qb."""
            wo3 = wo_sb.rearrange("p (n m) -> p n m", m=e)
            ot = opool.tile([128, e], F16, tag="ot", name="ot")
            si = qb * 4 + i
            for ob in range(2):
                ps = pp.tile([128, 512], F32, tag="pp", name="ps_o")
                for cc in range(ndq):
                    nc.tensor.matmul(
                        ps[:, :],
                        at_store[cc][:, i * 128 : (i + 1) * 128],
                        wo3[:, cc, ob * 512 : (ob + 1) * 512],
                        start=(cc == 0),
                        stop=(cc == ndq - 1),
                    )
                    pe_rows(512)
                if copy_eng is None:
                    nc.vector.tensor_copy(
                        ot[:, ob * 512 : (ob + 1) * 512], ps[:, :])
                else:
                    copy_eng.copy(ot[:, ob * 512 : (ob + 1) * 512], ps[:, :])
                nc.sync.dma_start(
                    out=out[si * 128 : (si + 1) * 128,
                            ob * 512 : (ob + 1) * 512],
                    in_=ot[:, ob * 512 : (ob + 1) * 512],
                )

        # ---- projection queue, deadline-ordered ----
        # Per window sb: q/k chunk c due just before head 2c; v slab due
        # during head 0's score groups (its diag PV needs it). Deadline key:
        # (sb, h_due) with v at h_due=1 (forced explicitly at h0's yields).
        proj_queue = []
        for sb in range(nwin):
            proj_queue.append((sb, 0, "q", sb, 0))
            proj_queue.append((sb, 0, "k", sb, 0))
            for ii in range(4):
                proj_queue.append((sb, 1, "v", sb, ii))
            for c in range(1, ndq):
                proj_queue.append((sb, 2 * c, "q", sb, c))
                proj_queue.append((sb, 2 * c, "k", sb, c))
        wo_queue = []

        def emit_proj_unit():
            _, _, kind, sb, j = proj_queue.pop(0)
            if kind == "q":
                proj_qk_unit(wq_sb, "q", q_sb, bq_sb, sb, j)
            elif kind == "k":
                proj_qk_unit(wk_sb, "k", k_sb, bk_sb, sb, j)
            else:
                proj_v_unit(sb, j)
            pe_rows(nec * SQ)

        def balance_filler(qb):
            # Keep PE fed while ACT is the pacing engine — but don't consume
            # units whose deadline lets them fill a FUTURE window's ACT-bound
            # stretch (they are the only legal filler there).
            if open_ps:
                return  # a phase-split unit owns a pp slot; don't rotate pp
            while eng_ns["pe"] < eng_ns["act"]:
                if proj_queue and (
                    (proj_queue[0][0], proj_queue[0][1]) < (qb + 1, 1)
                ):
                    emit_proj_unit()
                elif wo_queue:
                    wo_queue.pop(0)()
                else:
                    return

        def force_due(qb, h):
            while proj_queue and (proj_queue[0][0], proj_queue[0][1]) <= (qb, h):
                emit_proj_unit()

        def wo_full(qb, att_tiles, last=False):
            at_store = [None] * ndq
            for cc in range(ndq):
                wo_transpose_unit(att_tiles, cc, at_store)
            if debug:
                for qcl in range(4):
                    nc.sync.dma_start(
                        out=dbg_at[(qb * 4 + qcl) * 128 :
                                   (qb * 4 + qcl + 1) * 128, :],
                        in_=att_tiles[qcl][:, :],
                    )
            for i in range(4):
                # final window: ACT is idle by now, DVE is not
                wo_matmul_unit(at_store, qb, i,
                               copy_eng=nc.scalar if last else None)

        # ---- emission ----
        # bootstrap: the startup is DMA-bound; emit phase-split units in
        # A,A,B,B order so every unit's first contraction half runs while
        # the second DMA halves are still in flight
        boot = {("q", 0, 0), ("q", 0, 1), ("q", 0, 2), ("q", 0, 3),
                ("k", 0, 0), ("k", 0, 1), ("v", 0, 0), ("v", 0, 1),
                ("v", 0, 2), ("v", 0, 3)}
        for c0, c1 in ((0, 1), (2, 3)):
            proj_qk_phase(wq_sb, "q", q_sb, bq_sb, 0, c0, 0)
            proj_qk_phase(wq_sb, "q", q_sb, bq_sb, 0, c1, 0)
            proj_qk_phase(wq_sb, "q", q_sb, bq_sb, 0, c0, 1)
            proj_qk_phase(wq_sb, "q", q_sb, bq_sb, 0, c1, 1)
        proj_qk_phase(wk_sb, "k", k_sb, bk_sb, 0, 0, 0)
        proj_qk_phase(wk_sb, "k", k_sb, bk_sb, 0, 1, 0)
        proj_qk_phase(wk_sb, "k", k_sb, bk_sb, 0, 0, 1)
        proj_qk_phase(wk_sb, "k", k_sb, bk_sb, 0, 1, 1)
        proj_queue = [u for u in proj_queue if (u[2], u[3], u[4]) not in boot]

        prev = None  # deferred (qb, att_tiles, at_store) for wo
        last_store = [None] * ndq
        for qb in range(nwin):
            att_tiles = [
                atn.tile([128, dq], F16, tag=f"an{qcl}", name=f"an{qcl}_{qb}")
                for qcl in range(4)
            ]
            for h in range(hpc):
                force_due(qb, h)
                if h == 6 and qb + 1 < nwin:
                    # pre-force next window's first q/k chunks: the boundary
                    # head's scores start with zero projection latency
                    force_due(qb + 1, 0)
                yi = 0
                for _ in attention_head(qb, h, att_tiles):
                    yi += 1
                    if h == 0 and qb == 0:
                        # window 0's v slab is still streaming in: run the
                        # first contraction halves while the rest arrives
                        if yi == 1:
                            proj_v_phase(0, 0, 0)
                            proj_v_phase(0, 1, 0)
                        elif yi == 2:
                            proj_v_phase(0, 0, 1)
                            proj_v_phase(0, 1, 1)
                        elif yi == 3:
                            proj_v_phase(0, 2, 0)
                            proj_v_phase(0, 3, 0)
                            proj_v_phase(0, 2, 1)
                            proj_v_phase(0, 3, 1)
                    elif h == 0 and yi <= 2:
                        # v slab for this window's diagonal, 2 units per yield
                        for _ in range(2):
                            if proj_queue and proj_queue[0][2] == "v" \
                                    and proj_queue[0][3] == qb:
                                emit_proj_unit()
                    balance_filler(qb)
            # defer this window's Wo into the balance queue: it is the only
            # PE work with no deadline, so it belongs in the late ACT-bound
            # holes (atn/att bufs=4 make any emission order inversion-free)
            pqb, ptiles, pstore = qb, att_tiles, [None] * ndq

            def mk_tr(ptiles=ptiles, pstore=pstore, pqb=pqb):
                for cc in range(ndq):
                    wo_transpose_unit(ptiles, cc, pstore)
                if debug:
                    for qcl in range(4):
                        nc.sync.dma_start(
                            out=dbg_at[(pqb * 4 + qcl) * 128 :
                                       (pqb * 4 + qcl + 1) * 128, :],
                            in_=ptiles[qcl][:, :],
                        )

            if qb < nwin - 1:
                wo_queue.append(mk_tr)
                for i in range(4):
                    wo_queue.append(
                        lambda st=pstore, w=pqb, j=i: wo_matmul_unit(st, w, j))
            else:
                prev = (qb, att_tiles)
        while proj_queue:
            emit_proj_unit()
        while wo_queue:
            wo_queue.pop(0)()
        wo_full(*prev, last=True)

        if debug:
            for c in range(ndq):
                for w in range(nwin):
                    cs = slice(c * 128, (c + 1) * 128)
                    ws = slice(w * SQ, (w + 1) * SQ)
                    nc.sync.dma_start(out=dbg_q[cs, ws], in_=q_sb[c][w][:, :])
                    nc.sync.dma_start(out=dbg_k[cs, ws], in_=k_sb[c][w][:, :])
            for i in range(nsc):
                nc.sync.dma_start(
                    out=dbg_v[i * 128 : (i + 1) * 128, :], in_=v_sb[i][:, :]
                )

    if split_waits:
        split_excess_waits(nc)
    return nc


def make_crossmask():
    kk = np.arange(128)[:, None]
    qq = np.arange(128)[None, :]
    return np.where(kk <= qq, 0.0, NEG).astype(np.float16)


def classify_mask(mask):
    m = np.asarray(mask).reshape(S, S)
    if np.array_equal(m, np.tril(np.ones((S, S), bool))):
        return "causal"
    if m.all():
        return "dense"
    return "generic"


def prep_core_inputs(query, key, value, Wq, bq, Wk, bk, Wv, bv, Wo, bo, mask):
    """Shard + lay out host-side numpy inputs for the 8 cores."""
    kind = classify_mask(mask)
    maps = []
    for core in range(NCORES):
        b, gi = core // NGROUPS, core % NGROUPS
        gs = slice(gi * DQ, (gi + 1) * DQ)
        im = {
            "xq_t": np.ascontiguousarray(
                np.asarray(query[b]).T.astype(np.float16)),
            "xk_t": np.ascontiguousarray(
                np.asarray(key[b]).T.astype(np.float16)),
            "xv_t": np.ascontiguousarray(
                np.asarray(value[b]).T.astype(np.float16)),
            "wq_t": np.ascontiguousarray(
                np.asarray(Wq)[gs, :].T.astype(np.float16)),
            "wk_t": np.ascontiguousarray(
                np.asarray(Wk)[gs, :].T.astype(np.float16)),
            "wv_t": np.ascontiguousarray(
                np.asarray(Wv)[gs, :].T.astype(np.float16)),
            "wo_t": np.ascontiguousarray(
                np.asarray(Wo)[:, gs].T.astype(np.float16)),
            "consts_f32": np.ascontiguousarray(np.concatenate([
                np.asarray(bq)[gs].astype(np.float32).reshape(-1, 128).T,
                np.asarray(bk)[gs].astype(np.float32).reshape(-1, 128).T,
                np.broadcast_to(
                    np.asarray(bv)[gs].astype(np.float32), (128, DQ)),
            ], axis=1)),
            "consts_f16": np.ascontiguousarray(np.concatenate([
                np.eye(128, dtype=np.float16), make_crossmask()
            ], axis=1)),
        }
        maps.append(im)
    return maps, kind


def make_runner(nc, n_cores=NCORES):
    """Build a reusable jitted SPMD executor for `nc` on cores 0..n_cores-1."""
    import jax
    from jax.experimental.shard_map import shard_map
    from jax.sharding import Mesh, PartitionSpec

    from concourse import bass2jax, mybir as _mybir

    bass2jax.install_neuronx_cc_hook()

    partition_name = (
        nc.partition_id_tensor.name if nc.partition_id_tensor else None
    )
    in_names, out_names, out_avals, zero_shapes = [], [], [], []
    for alloc in nc.m.functions[0].allocations:
        if not isinstance(alloc, _mybir.MemoryLocationSet):
            continue
        name = alloc.memorylocations[0].name
        if alloc.kind == "ExternalInput":
            if name != partition_name:
                in_names.append(name)
        elif alloc.kind == "ExternalOutput":
            out_names.append(name)
            shape = tuple(alloc.tensor_shape)
            dtype = _mybir.dt.np(alloc.dtype)
            out_avals.append(jax.core.ShapedArray(shape, dtype))
            zero_shapes.append((shape, dtype))
    n_params = len(in_names)
    all_in = list(in_names) + list(out_names)
    if partition_name is not None:
        all_in.append(partition_name)

    def _body(*args):
        operands = list(args)
        if partition_name is not None:
            operands.append(bass2jax.partition_id_tensor())
        outs = bass2jax._bass_exec_p.bind(
            *operands,
            out_avals=tuple(out_avals),
            in_names=tuple(all_in),
            out_names=tuple(out_names),
            lowering_input_output_aliases=(),
            sim_require_finite=True,
            sim_require_nnan=True,
            nc=nc,
        )
        return tuple(outs)

    devices = jax.devices()[:n_cores]
    assert len(devices) == n_cores
    mesh = Mesh(np.asarray(devices), ("core",))
    in_specs = (PartitionSpec("core"),) * (n_params + len(out_names))
    out_specs = (PartitionSpec("core"),) * len(out_names)
    sharded = jax.jit(
        shard_map(
            _body,
            mesh=mesh,
            in_specs=in_specs,
            out_specs=out_specs,
            check_rep=False,
        ),
        keep_unused=True,
    )
    zeros = [
        np.zeros((n_cores * sh[0], *sh[1:]), dt) for sh, dt in zero_shapes
    ]

    def concat_inputs(in_maps):
        return [
            np.concatenate(
                [np.asarray(in_maps[c][n]) for c in range(n_cores)], axis=0
            )
            for n in in_names
        ]

    def run(in_maps):
        out_arrs = sharded(*concat_inputs(in_maps), *zeros)
        return [
            {
                name: np.asarray(out_arrs[i]).reshape(
                    n_cores, *out_avals[i].shape
                )[c]
                for i, name in enumerate(out_names)
            }
            for c in range(n_cores)
        ]

    run.sharded = sharded
    run.concat_inputs = concat_inputs
    run.zeros = zeros
    run.out_names = out_names
    run.out_avals = out_avals
    return run


_CACHE = {}


def get_runner(kind="causal"):
    if kind not in _CACHE:
        nc = build_kernel(causal=(kind == "causal"))
        _CACHE[kind] = make_runner(nc)
    return _CACHE[kind]


def _numpy_reference(query, key, value, Wq, bq, Wk, bk, Wv, bv, Wo, bo, mask):
    q = (query @ Wq.T + bq).reshape(B, S, H, D).transpose(0, 2, 1, 3)
    k = (key @ Wk.T + bk).reshape(B, S, H, D).transpose(0, 2, 1, 3)
    v = (value @ Wv.T + bv).reshape(B, S, H, D).transpose(0, 2, 1, 3)
    sc = np.einsum("bhqd,bhkd->bhqk", q, k) / np.sqrt(D)
    sc = np.where(np.asarray(mask).reshape(1, 1, S, S), sc, -np.inf)
    sc -= sc.max(axis=-1, keepdims=True)
    p = np.exp(sc)
    p /= p.sum(axis=-1, keepdims=True)
    o = np.einsum("bhqk,bhkd->bhqd", p, v)
    o = o.transpose(0, 2, 1, 3).reshape(B, S, E)
    return o @ Wo.T + bo


def kernel(**inputs) -> np.ndarray:
    kind = classify_mask(inputs["mask"])
    if kind == "generic":
        fp = {k: np.asarray(v, np.float32) for k, v in inputs.items()
              if k != "mask"}
        return _numpy_reference(mask=inputs["mask"], **fp).astype(np.float32)
    in_maps, kind = prep_core_inputs(**inputs)
    run = get_runner(kind)
    results = run(in_maps)
    bo = np.asarray(inputs["bo"], dtype=np.float32)
    out = np.empty((B, S, E), dtype=np.float32)
    for b in range(B):
        acc = results[b * NGROUPS]["out"].astype(np.float32)
        for gi in range(1, NGROUPS):
            acc = acc + results[b * NGROUPS + gi]["out"].astype(np.float32)
        out[b] = acc + bo[None, :]
    return out



# revision 37
# speedup vs baseline: 1.0771x; 1.0771x over previous
"""Trainium2 Bass kernel: 16-head causal attention (B=4, S=2048, E=1024).

Sharding: 8 cores = 4 batches x 2 head-groups (8 heads each); host sums the
two head-group partials (fp32) and adds bo.

Per-core pipeline (PSUM accumulates fp32):
  - projections run as all-fp8e4m3 DoubleRow matmuls (0.5 cyc/row, two
    128-deep k-tiles per instruction). Host ships x and 64*W pre-split into
    (hi, lo) fp8 pairs; the three products hi*hi + hi*lo + lo*hi reconstruct
    the f32 product to ~0.1% (the 64x weight scale keeps W_lo out of fp8
    subnormals). 12 DR matmuls replace 8 fp16 matmuls per unit: 1.33x.
    q^T = Wq_g X^T, k^T = Wk_g X^T  ([dq, S] f16, carrying the 64x scale)
    V   = X^T.T Wv_g^T              (natural [S, dv] bf16, also 64x-scaled;
                                     the denominator ones-column is 64.0 so
                                     normalization cancels the scale exactly)
  - scores^T[k, q] in f16 at 128x128 causal granularity (fp8 would add
    ~2.6% exp-space noise; fully-masked sub-blocks are skipped). The 64x64
    scale folds into the exp scale: P^T = exp(scores^T / (8*4096)) on ACT
    -> bf16. Diagonal-crossing sub-blocks are masked AFTER the exp by a DVE
    multiply with a 0/1 lower-triangle tile (frees the PE mask matmuls).
  - PV: out[q, 65] += P^T_block^T V_aug: stationary = P^T [128,128] bf16,
    moving = V_aug [128,65] bf16 -> 65 rows/block. One PSUM accumulation
    group per vpa bank (single start/stop; sub-regions auto-initialize via
    the pending-zero mechanism). (fp8 DR is impossible here: P's dynamic
    range spans e^18 and e4m3 covers only ~e^12.)
  - normalize: DVE reciprocal of the denominator column + tensor_scalar_mul
  - attn [q, dq] f16 -> PE-transpose [dq, q] -> Wo matmul -> f16 partials
Scheduling: the emitter interleaves projection/output-projection work into the
ACT-bound attention windows (deadline queue + PE-vs-ACT balance heuristic),
batches DMAs into ~45 large transfers, and software-pipelines scores/exp/PV
with a lag of one exp group.
"""

import contextlib

import numpy as np

import bass_rust
import concourse.bass as bass
import concourse.mybir as mybir
import concourse.tile as tile

F32 = mybir.dt.float32
F16 = mybir.dt.float16
BF16 = mybir.dt.bfloat16
F8 = mybir.dt.float8e4
NP_F8 = mybir.dt.np(F8)
AF = mybir.ActivationFunctionType
DRMODE = mybir.MatmulPerfMode.DoubleRow

B, S, E = 4, 2048, 1024
H, D = 16, 64
NCORES = 8
NGROUPS = 2            # head groups (tensor parallel)
HPC = H // NGROUPS     # heads per core
DQ = HPC * D           # per-core projection width = 512
WSCALE = 64.0          # weight pre-scale keeping W_lo fp8-normal

ACT_FUDGE = 1.0        # filler-balance bias: estimated ACT ns multiplier
ROT = 2                # head group-rotation depth
BAL_BIAS = 0.0         # extra ns of PE filler allowed past the ACT estimate
PREF_H = 7             # head at which next window's first chunks pre-force

SK = 128               # k sub-block (partition dim of scores^T)
SQ = 512               # q window
GW = 1024              # exp group width (psum [128, GW])


def split_excess_waits(nc, maxw=1):
    """This container's walrus supports one sem wait per instruction;
    hoist extras onto same-engine nops just before the instruction."""
    n_new = 0
    for bb in nc.main_func.blocks:
        new_list = []
        changed = False
        for inst in list(bb.instructions):
            si = inst.sync_info
            waits = list(si.on_wait) if si and si.on_wait else []
            if len(waits) > maxw:
                changed = True
                extra, keep = waits[:-maxw], waits[-maxw:]
                for ci in range(0, len(extra), maxw):
                    nop = bass_rust.InstNoOp(
                        name=f"I-waitsplit-{n_new}", ins=[], outs=[]
                    )
                    n_new += 1
                    nop.engine = inst.engine
                    nop.sync_info = mybir.SyncInfo(
                        on_wait=extra[ci : ci + maxw], on_update=[]
                    )
                    new_list.append(nop)
                inst.sync_info = mybir.SyncInfo(
                    on_wait=keep,
                    on_update=list(si.on_update) if si.on_update else [],
                )
            new_list.append(inst)
        if changed:
            bb.instructions = new_list
    return n_new


def matmul_dr(nc, out, lhsT, rhs, start, stop):
    """fp8 DoubleRow matmul: out += sum_i lhsT[:, i, :].T @ rhs[:, i, :]
    for the two k-tiles i. (nc.tensor.matmul minus its both-shapes-halved
    bookkeeping; operands are [K, 2, M] / [K, 2, N] fp8 APs.)"""
    eng = nc.tensor
    keep_dims = {0, 1}
    ifmap_ap = eng.lower_ap(rhs.opt(keep_dims), opt=False)
    weights_ap = eng.lower_ap(lhsT.opt(keep_dims), opt=False,
                              for_matmul_weights=True)
    out_ap = eng.lower_ap(out)
    return eng.add_instruction(
        mybir.InstMatmult(
            name=nc.get_next_instruction_name(),
            replication_resolution=0,
            replication_shift_amnt=0,
            replication_num_rows=0,
            start_tensor_calc=start,
            stop_tensor_calc=stop,
            ins=[ifmap_ap, weights_ap],
            outs=[out_ap],
            perf_mode=DRMODE,
            is_transpose=None,
            ifmap_quant_offset=None,
            weights_quant_offset=None,
            bass_skip_group_check=False,
            tile_position=(lhsT.base_partition(), out.base_partition()),
            tile_size=(128, 128),
        )
    )


def build_kernel(causal=True, split_waits=True, debug=False):
    s, e, hpc, d = S, E, HPC, D
    dq = hpc * d              # 512
    nec = e // 128            # 8 input-feature chunks
    ndq = dq // 128           # 4 projection partition chunks
    nwin = s // SQ            # 4 q windows
    nsc = s // 128            # 16 s chunks

    nc = bass.Bass()

    # x and 64*W ship as fp8 (hi, lo) splits: x per window [e, nwin, 2, SQ],
    # W per e-row [e, 2, dq] -- same byte counts as the old f16 tensors.
    xq = nc.declare_dram_parameter("xq8", [e, 2 * s], F8, isOutput=False)
    xk = nc.declare_dram_parameter("xk8", [e, 2 * s], F8, isOutput=False)
    xv = nc.declare_dram_parameter("xv8", [e, 2 * s], F8, isOutput=False)
    wqd = nc.declare_dram_parameter("wq8", [e, 2 * dq], F8, isOutput=False)
    wkd = nc.declare_dram_parameter("wk8", [e, 2 * dq], F8, isOutput=False)
    wvd = nc.declare_dram_parameter("wv8", [e, 2 * dq], F8, isOutput=False)
    wod = nc.declare_dram_parameter("wo_t", [dq, e], F16, isOutput=False)
    # packed constants: [bq(4) | bk(4) | bv_b(512)] f32 (64x-scaled),
    # [ident | 2^15*ident | crossmask(-2^15/0)] f16: the additive causal
    # mask for diagonal sub-blocks is (2^15 I)(-2^15 mask) = -2^30, which
    # survives the 64x64 score scale (f16 alone caps at -65504)
    cfd = nc.declare_dram_parameter("consts_f32", [128, 2 * ndq + dq], F32,
                                    isOutput=False)
    chd = nc.declare_dram_parameter("consts_f16", [128, 384], F16,
                                    isOutput=False)
    out = nc.declare_dram_parameter("out", [s, e], F16, isOutput=True)
    if debug:
        dbg_q = nc.declare_dram_parameter("dbg_q", [dq, s], F16, isOutput=True)
        dbg_k = nc.declare_dram_parameter("dbg_k", [dq, s], F16, isOutput=True)
        dbg_v = nc.declare_dram_parameter(
            "dbg_v", [s, hpc * (d + 1)], BF16, isOutput=True
        )
        dbg_at = nc.declare_dram_parameter("dbg_at", [s, dq], F16, isOutput=True)
        dbg_pt = nc.declare_dram_parameter("dbg_pt", [128, 17408], BF16,
                                           isOutput=True)
        dbg_rc = nc.declare_dram_parameter("dbg_rc", [128, 16], F32,
                                           isOutput=True)
        dbg_off = [0]

    with tile.TileContext(nc) as tc, contextlib.ExitStack() as ctx:
        pers = ctx.enter_context(tc.tile_pool(name="pers", bufs=1))
        xpool = ctx.enter_context(tc.tile_pool(name="xp", bufs=3))
        ppool = ctx.enter_context(tc.tile_pool(name="ppl", bufs=4))
        atn = ctx.enter_context(tc.tile_pool(name="atn", bufs=4))
        att = ctx.enter_context(tc.tile_pool(name="att", bufs=4))
        nrm = ctx.enter_context(tc.tile_pool(name="nrm", bufs=4))
        opool = ctx.enter_context(tc.tile_pool(name="opl", bufs=3))
        pp = ctx.enter_context(tc.tile_pool(name="pp", bufs=3, space="PSUM"))
        sp = ctx.enter_context(tc.tile_pool(name="sp", bufs=2, space="PSUM"))
        vp = ctx.enter_context(tc.tile_pool(name="vp", bufs=1, space="PSUM"))

        # ---- persistent tensors ----
        cf_sb = pers.tile([128, 2 * ndq + dq], F32, name="cf_sb")
        ch_sb = pers.tile([128, 384], F16, name="ch_sb")
        bq_sb = cf_sb[:, 0:ndq]
        bk_sb = cf_sb[:, ndq : 2 * ndq]
        bv_sb = cf_sb[:, 2 * ndq : 2 * ndq + dq]
        id_sb = ch_sb[:, 0:128]
        id2_sb = ch_sb[:, 128:256]  # 2^15 * identity
        mk_sb = ch_sb[:, 256:384]   # -2^15 where k > q, else 0
        q_sb = [
            [pers.tile([128, SQ], F16, name=f"q_sb{c}_{w}") for w in range(nwin)]
            for c in range(ndq)
        ]
        k_sb = [
            [pers.tile([128, SQ], F16, name=f"k_sb{c}_{w}") for w in range(nwin)]
            for c in range(ndq)
        ]
        v_sb = [
            pers.tile([128, hpc * (d + 1)], BF16, name=f"v_sb{i}")
            for i in range(nsc)
        ]
        wq_sb = pers.tile([128, nec * 2 * dq], F8, name="wq_sb")
        wk_sb = pers.tile([128, nec * 2 * dq], F8, name="wk_sb")
        wv_sb = pers.tile([128, nec * 2 * dq], F8, name="wv_sb")
        wo_sb = pers.tile([128, ndq * e], F16, name="wo_sb")

        # ---- DMA helpers (SP engine -> one HWDGE queue, program order) ----
        def load_w_part(wt, dst, part, nparts=2):
            # e-chunk group `part` of [e, 2*dq] -> dst cols
            g = nec // nparts
            src = wt.rearrange("(n p) m -> p n m", p=128)
            nc.sync.dma_start(
                out=dst.rearrange("p (n m) -> p n m", m=2 * dq)[
                    :, part * g : (part + 1) * g, :
                ],
                in_=src[:, part * g : (part + 1) * g, :],
            )

        def load_x_slab(xt, dst, sb, part=None, nparts=2):
            # dst: [128, nec*2*512] tile; window sb's (hi|lo) 1024-col slab
            src = xt.rearrange("(n p) m -> p n m", p=128)
            d3 = dst.rearrange("p (n m) -> p n m", m=2 * SQ)
            if part is None:
                nc.sync.dma_start(
                    out=d3[:, :, :],
                    in_=src[:, :, sb * 2 * SQ : (sb + 1) * 2 * SQ],
                )
            else:
                g = nec // nparts
                nc.sync.dma_start(
                    out=d3[:, part * g : (part + 1) * g, :],
                    in_=src[:, part * g : (part + 1) * g,
                            sb * 2 * SQ : (sb + 1) * 2 * SQ],
                )



        x_t = {}  # (tensor, sb) -> slab tile
        for t, xd in (("q", xq), ("k", xk), ("v", xv)):
            x_t[t, 0] = xpool.tile([128, nec * 2 * SQ], F8, tag=f"x{t}",
                                   name=f"x{t}0", bufs=3)
        # slab 0 interleaved with weight pieces for earliest unblock;
        # wq/xq0 in quarters so the first projection matmuls start ASAP
        for part in range(4):
            load_w_part(wqd, wq_sb, part, nparts=4)
            load_x_slab(xq, x_t["q", 0], 0, part=part, nparts=4)
        # packed constants (biases for the first bias-add, mask for h0 scores)
        nc.sync.dma_start(out=cf_sb[:, :], in_=cfd[:, :])
        nc.sync.dma_start(out=ch_sb[:, :], in_=chd[:, :])
        load_w_part(wkd, wk_sb, 0)
        load_x_slab(xk, x_t["k", 0], 0, part=0)
        load_w_part(wkd, wk_sb, 1)
        load_x_slab(xk, x_t["k", 0], 0, part=1)
        load_w_part(wvd, wv_sb, 0)
        load_x_slab(xv, x_t["v", 0], 0, part=0)
        load_w_part(wvd, wv_sb, 1)
        load_x_slab(xv, x_t["v", 0], 0, part=1)
        x_t["q", 1] = xpool.tile([128, nec * 2 * SQ], F8, tag="xq",
                                 name="xq1", bufs=3)
        load_x_slab(xq, x_t["q", 1], 1)
        for sb in range(1, nwin):
            for t, xd in (("q", xq), ("k", xk), ("v", xv)):
                if (t, sb) in x_t:
                    continue
                x_t[t, sb] = xpool.tile([128, nec * 2 * SQ], F8, tag=f"x{t}",
                                        name=f"x{t}{sb}", bufs=3)
                load_x_slab(xd, x_t[t, sb], sb)
            if sb == 1:
                nc.sync.dma_start(
                    out=wo_sb.rearrange("p (n m) -> p n m", m=e),
                    in_=wod.rearrange("(n p) m -> p n m", p=128),
                )

        # denominator columns of v_sb carry the 64x v-scale so the
        # normalize reciprocal cancels it; once, on the idle gpsimd engine
        for i in range(nsc):
            v3 = v_sb[i].rearrange("p (h t) -> p h t", t=d + 1)
            nc.gpsimd.memset(v3[:, :, d], WSCALE)

        # ---- compute unit generators ----
        # fp8 split-product term order: hi*hi, hi*lo, lo*hi (lo*lo ~ 1e-3
        # relative, dropped). Each term runs as nec/2 DoubleRow matmuls
        # pairing adjacent e-chunks: 12 DRs replace 8 f16 matmuls.
        TERMS = ((0, 0), (0, 1), (1, 0))
        NDR = 3 * (nec // 2)

        def w4(wt):
            return wt.rearrange("p (n t m) -> p n t m", t=2, m=dq)

        def xs4(xt, sb):
            return x_t[xt, sb].rearrange("p (n t m) -> p n t m", t=2, m=SQ)

        def qk_drs(w_sb_t, xt, sb, c, ps, drs):
            wv_ = w4(w_sb_t)
            xv_ = xs4(xt, sb)
            cs = slice(c * 128, (c + 1) * 128)
            for n in drs:
                wt, xt_ = TERMS[n // (nec // 2)]
                j = n % (nec // 2)
                matmul_dr(
                    nc, ps[:, :],
                    wv_[:, 2 * j : 2 * j + 2, wt, cs],
                    xv_[:, 2 * j : 2 * j + 2, xt_, :],
                    start=(n == 0), stop=(n == NDR - 1),
                )
            pe_rows(len(drs) * SQ // 2)

        def v_drs(sb, ii, ps, drs):
            wv_ = w4(wv_sb)
            xv_ = xs4("v", sb)
            for n in drs:
                wt, xt_ = TERMS[n // (nec // 2)]
                j = n % (nec // 2)
                matmul_dr(
                    nc, ps[:, :],
                    xv_[:, 2 * j : 2 * j + 2, xt_, ii * 128 : ii * 128 + 128],
                    wv_[:, 2 * j : 2 * j + 2, wt, :],
                    start=(n == 0), stop=(n == NDR - 1),
                )
            pe_rows(len(drs) * dq // 2)

        open_ps = {}

        def proj_qk_phase(w_sb_t, xt, dst, bias, sb, c, phase):
            """Half of a q/k projection unit. Phase 0 allocates the psum tile
            and runs the DRs touching e-chunk pairs 0,1; phase 1 finishes
            pairs 2,3 and applies the bias. Between a unit's phases at most
            one other pp allocation may occur (pp bufs=2)."""
            key = ("qk", xt, sb, c)
            halves = ([n for n in range(NDR) if n % (nec // 2) < 2],
                      [n for n in range(NDR) if n % (nec // 2) >= 2])
            if phase == 0:
                ps = pp.tile([128, SQ], F32, tag="pp", name="ps_pj")
                open_ps[key] = ps
            else:
                ps = open_ps.pop(key)
            qk_drs(w_sb_t, xt, sb, c, ps, halves[phase])
            if phase == 1:
                nc.vector.tensor_scalar_add(
                    dst[c][sb][:, :], ps[:, :], bias[:, c : c + 1]
                )

        def proj_v_phase(sb, ii, phase):
            key = ("v", sb, ii)
            halves = ([n for n in range(NDR) if n % (nec // 2) < 2],
                      [n for n in range(NDR) if n % (nec // 2) >= 2])
            if phase == 0:
                ps = pp.tile([128, dq], F32, tag="pp", name="ps_v")
                open_ps[key] = ps
            else:
                ps = open_ps.pop(key)
            v_drs(sb, ii, ps, halves[phase])
            if phase == 1:
                i = sb * 4 + ii
                v3 = v_sb[i].rearrange("p (h t) -> p h t", t=d + 1)
                nc.vector.tensor_add(
                    v3[:, :, 0:d],
                    ps[:, :].rearrange("p (h t) -> p h t", t=d),
                    bv_sb[:, :].rearrange("p (h t) -> p h t", t=d),
                )

        def proj_qk_unit(w_sb_t, xt, dst, bias, sb, c):
            """One [128,512] slab-column of a transposed projection."""
            ps = pp.tile([128, SQ], F32, tag="pp", name="ps_pj")
            qk_drs(w_sb_t, xt, sb, c, ps, range(NDR))
            nc.vector.tensor_scalar_add(
                dst[c][sb][:, :], ps[:, :], bias[:, c : c + 1]
            )

        def proj_v_unit(sb, ii):
            """One [128(s), dq] natural-layout V chunk (i = sb*4+ii)."""
            i = sb * 4 + ii
            ps = pp.tile([128, dq], F32, tag="pp", name="ps_v")
            v_drs(sb, ii, ps, range(NDR))
            v3 = v_sb[i].rearrange("p (h t) -> p h t", t=d + 1)
            nc.vector.tensor_add(
                v3[:, :, 0:d],
                ps[:, :].rearrange("p (h t) -> p h t", t=d),
                bv_sb[:, :].rearrange("p (h t) -> p h t", t=d),
            )

        # static PE/ACT occupancy estimate driving filler insertion
        eng_ns = {"pe": 0.0, "act": 0.0}

        def pe_rows(n):
            eng_ns["pe"] += n * 0.4167

        def act_cols(n):
            eng_ns["act"] += ACT_FUDGE * (n * 0.8333 + 185.0)  # tuned filler bias

        def attention_head(qb, h, att_tiles, pre_last_cb=None,
                           act_norm=False, lag=2):
            """scores+exp+PV+normalize for one (window, head).

            Generator: yields after each score-group / PV emission so the
            driver can interleave PE filler while ACT churns through exps.
            pre_last_cb: emitted right after the last score group (tail
            shortening for the final head). act_norm: do half the normalize
            multiplies on ACT (only sensible when ACT is idle afterwards).
            """
            c, hp = h // 2, (h % 2) * 64
            nkb = 4 * qb + 4 if causal else nsc
            # segments: (kb, qstart_global, width)
            segs = []
            for kb in range(nkb):
                if causal and kb >= 4 * qb:
                    qs = kb * 128
                else:
                    qs = qb * SQ
                segs.append((kb, qs, (qb + 1) * SQ - qs))
            # greedy-pack into exp groups of width <= GW
            groups, cur, curw = [], [], 0
            for seg in segs:
                if curw + seg[2] > GW:
                    groups.append(cur)
                    cur, curw = [], 0
                cur.append(seg)
                curw += seg[2]
            if cur:
                groups.append(cur)
            if len(groups) > 1 and ROT:
                # smallest group first: its short exp lands while ACT still
                # drains the previous head, instead of bubbling at head end
                groups = groups[-ROT:] + groups[:-ROT]

            vpa = vp.tile([128, 4 * (d + 1)], F32, tag="vo", name="vpa")
            last_kb = nkb - 1
            npv = sum(
                1 for kb in range(nkb) for qcl in range(4)
                if not (causal and 4 * qb + qcl < kb))
            pv_n = [0]

            def emit_scores(grp):
                gw = sum(g[2] for g in grp)
                scp = sp.tile([128, GW], F32, tag="sc", name="scp")
                off = 0
                for kb, qs, w in grp:
                    ks = k_sb[c][kb // 4][hp : hp + d,
                                          (kb % 4) * 128 : (kb % 4) * 128 + 128]
                    qw_ = q_sb[c][qs // SQ]
                    if causal and kb >= 4 * qb:
                        # additive -2^30 mask for the diagonal-crossing
                        # sub-block ((2^15 I) @ (-2^15 crossmask)): in-group
                        # on PE, so no cross-engine hop before the PV
                        nc.tensor.matmul(scp[:, off : off + 128], id2_sb[:, :],
                                         mk_sb[:, :], start=True, stop=False)
                        nc.tensor.matmul(
                            scp[:, off : off + 128], ks,
                            qw_[hp : hp + d, qs % SQ : qs % SQ + 128],
                            start=False, stop=True,
                        )
                        pe_rows(256)
                        if w > 128:
                            nc.tensor.matmul(
                                scp[:, off + 128 : off + w], ks,
                                qw_[hp : hp + d, qs % SQ + 128 : qs % SQ + w],
                                start=True, stop=True,
                            )
                            pe_rows(w - 128)
                    else:
                        nc.tensor.matmul(
                            scp[:, off : off + w], ks,
                            qw_[hp : hp + d, qs % SQ : qs % SQ + w],
                            start=True, stop=True,
                        )
                        pe_rows(w)
                    off += w
                pt = ppool.tile([128, GW], BF16, tag="pt", name="pt")
                nc.scalar.activation(
                    pt[:, 0:gw], scp[:, 0:gw], AF.Exp,
                    scale=float(1.0 / (np.sqrt(d) * WSCALE * WSCALE)),
                )
                act_cols(gw)
                if debug and h == 0:
                    nc.sync.dma_start(
                        out=dbg_pt[:, dbg_off[0] : dbg_off[0] + gw],
                        in_=pt[:, 0:gw])
                    dbg_off[0] += gw
                return pt

            def emit_pv(grp, pt):
                # One psum accumulation group for the whole vpa bank: a
                # start marks the full 2KB zero-region pending-zero, so only
                # the first matmul may carry start and only the last stop;
                # each sub-region auto-initializes on its first write.
                off = 0
                for kb, qs, w in grp:
                    for qcl in range(4):
                        qg = 4 * qb + qcl           # global q chunk
                        if causal and qg < kb:
                            continue                 # fully masked block
                        boff = off + qcl * 128 + qb * SQ - qs
                        nc.tensor.matmul(
                            vpa[:, qcl * (d + 1) : (qcl + 1) * (d + 1)],
                            pt[:, boff : boff + 128],
                            v_sb[kb][:, h * (d + 1) : (h + 1) * (d + 1)],
                            start=(pv_n[0] == 0),
                            stop=(pv_n[0] == npv - 1),
                        )
                        pv_n[0] += 1
                        pe_rows(d + 1)
                    off += w

            # lag-N software pipeline: scores g+N overlap exp of g. The
            # (qb0, h0) bootstrap needs lag=1 so its yi==3 hook (the window-0
            # v units that pv(g1) reads) still fires before the pend drain.
            pend = []
            for gi, grp in enumerate(groups):
                pt = emit_scores(grp)
                if pre_last_cb is not None and gi == len(groups) - 1:
                    pre_last_cb()
                yield
                pend.append((grp, pt))
                if len(pend) > lag:
                    emit_pv(*pend.pop(0))
                    yield
            for item in pend:
                emit_pv(*item)

            v4 = vpa.rearrange("p (qc t) -> p qc t", t=d + 1)
            rcp = nrm.tile([128, 4], F32, tag="rcp", name="rcp")
            nc.vector.reciprocal(rcp[:, :], v4[:, :, d])
            if debug and h == 0:
                nc.sync.dma_start(out=dbg_rc[:, qb * 4 : qb * 4 + 4],
                                  in_=rcp[:, :])
            for qcl in range(4):
                if act_norm and qcl >= 2:
                    nc.scalar.activation(
                        att_tiles[qcl][:, h * d : (h + 1) * d],
                        v4[:, qcl, 0:d],
                        AF.Copy,
                        scale=rcp[:, qcl : qcl + 1],
                    )
                else:
                    nc.vector.tensor_scalar_mul(
                        att_tiles[qcl][:, h * d : (h + 1) * d],
                        v4[:, qcl, 0:d],
                        rcp[:, qcl : qcl + 1],
                    )

        def wo_transpose_unit(att_tiles, cc, at_store, copy_eng=None):
            """Transpose attn chunk cc (heads 2cc, 2cc+1) -> at_store[cc]."""
            tp = pp.tile([128, SQ], F16, tag="pp", name="tp")
            for qcl in range(4):
                nc.tensor.transpose(
                    tp[:, qcl * 128 : (qcl + 1) * 128],
                    att_tiles[qcl][:, cc * 128 : (cc + 1) * 128],
                    id_sb[:, :],
                )
                pe_rows(128)
            at_ = att.tile([128, SQ], F16, tag=f"at{cc}", name="at_")
            if copy_eng is None:
                nc.vector.tensor_copy(at_[:, :], tp[:, :])
            else:
                copy_eng.copy(at_[:, :], tp[:, :])
            at_store[cc] = at_

        def wo_matmul_unit(at_store, qb, i, copy_eng=None):
            """Output projection + store for s-chunk i of window qb."""
            wo3 = wo_sb.rearrange("p (n m) -> p n m", m=e)
            ot = opool.tile([128, e], F16, tag="ot", name="ot")
            si = qb * 4 + i
            for ob in range(2):
                ps = pp.tile([128, 512], F32, tag="pp", name="ps_o")
                for cc in range(ndq):
                    nc.tensor.matmul(
                        ps[:, :],
                        at_store[cc][:, i * 128 : (i + 1) * 128],
                        wo3[:, cc, ob * 512 : (ob + 1) * 512],
                        start=(cc == 0),
                        stop=(cc == ndq - 1),
                    )
                    pe_rows(512)
                if copy_eng is None:
                    nc.vector.tensor_copy(
                        ot[:, ob * 512 : (ob + 1) * 512], ps[:, :])
                else:
                    copy_eng.copy(ot[:, ob * 512 : (ob + 1) * 512], ps[:, :])
                nc.sync.dma_start(
                    out=out[si * 128 : (si + 1) * 128,
                            ob * 512 : (ob + 1) * 512],
                    in_=ot[:, ob * 512 : (ob + 1) * 512],
                )

        # ---- projection queue, deadline-ordered ----
        # Per window sb: q/k chunk c due just before head 2c; v slab due
        # during head 0's score groups (its diag PV needs it). Deadline key:
        # (sb, h_due) with v at h_due=1 (forced explicitly at h0's yields).
        proj_queue = []
        for sb in range(nwin):
            proj_queue.append((sb, 0, "q", sb, 0))
            proj_queue.append((sb, 0, "k", sb, 0))
            for ii in range(4):
                proj_queue.append((sb, 1, "v", sb, ii))
            for c in range(1, ndq):
                proj_queue.append((sb, 2 * c, "q", sb, c))
                proj_queue.append((sb, 2 * c, "k", sb, c))
        wo_queue = []

        def emit_proj_unit():
            _, _, kind, sb, j = proj_queue.pop(0)
            if kind == "q":
                proj_qk_unit(wq_sb, "q", q_sb, bq_sb, sb, j)
            elif kind == "k":
                proj_qk_unit(wk_sb, "k", k_sb, bk_sb, sb, j)
            else:
                proj_v_unit(sb, j)

        def balance_filler(qb):
            # Keep PE fed while ACT is the pacing engine — but don't consume
            # units whose deadline lets them fill a FUTURE window's ACT-bound
            # stretch (they are the only legal filler there).
            if open_ps:
                return  # a phase-split unit owns a pp slot; don't rotate pp
            while eng_ns["pe"] < eng_ns["act"] + BAL_BIAS:
                if proj_queue and (
                    (proj_queue[0][0], proj_queue[0][1]) < (qb + 1, 1)
                ):
                    emit_proj_unit()
                elif wo_queue:
                    wo_queue.pop(0)()
                else:
                    return

        def force_due(qb, h):
            while proj_queue and (proj_queue[0][0], proj_queue[0][1]) <= (qb, h):
                emit_proj_unit()

        def wo_full(qb, att_tiles, last=False):
            at_store = [None] * ndq
            for cc in range(ndq):
                wo_transpose_unit(att_tiles, cc, at_store)
            if debug:
                for qcl in range(4):
                    nc.sync.dma_start(
                        out=dbg_at[(qb * 4 + qcl) * 128 :
                                   (qb * 4 + qcl + 1) * 128, :],
                        in_=att_tiles[qcl][:, :],
                    )
            for i in range(4):
                # final window: ACT is idle by now, DVE is not
                wo_matmul_unit(at_store, qb, i,
                               copy_eng=nc.scalar if last else None)

        # ---- emission ----
        # bootstrap: the startup is DMA-bound; emit phase-split units in
        # A,A,B,B order so every unit's first contraction half runs while
        # the second DMA halves are still in flight
        boot = {("q", 0, 0), ("q", 0, 1), ("q", 0, 2), ("q", 0, 3),
                ("k", 0, 0), ("k", 0, 1), ("v", 0, 0), ("v", 0, 1),
                ("v", 0, 2), ("v", 0, 3)}
        for c0, c1 in ((0, 1), (2, 3)):
            proj_qk_phase(wq_sb, "q", q_sb, bq_sb, 0, c0, 0)
            proj_qk_phase(wq_sb, "q", q_sb, bq_sb, 0, c1, 0)
            proj_qk_phase(wq_sb, "q", q_sb, bq_sb, 0, c0, 1)
            proj_qk_phase(wq_sb, "q", q_sb, bq_sb, 0, c1, 1)
        proj_qk_phase(wk_sb, "k", k_sb, bk_sb, 0, 0, 0)
        proj_qk_phase(wk_sb, "k", k_sb, bk_sb, 0, 1, 0)
        proj_qk_phase(wk_sb, "k", k_sb, bk_sb, 0, 0, 1)
        proj_qk_phase(wk_sb, "k", k_sb, bk_sb, 0, 1, 1)
        proj_queue = [u for u in proj_queue if (u[2], u[3], u[4]) not in boot]

        prev = None  # deferred (qb, att_tiles, at_store) for wo
        last_store = [None] * ndq
        for qb in range(nwin):
            att_tiles = [
                atn.tile([128, dq], F16, tag=f"an{qcl}", name=f"an{qcl}_{qb}")
                for qcl in range(4)
            ]
            for h in range(hpc):
                force_due(qb, h)
                if h == PREF_H and qb + 1 < nwin:
                    # pre-force next window's first q/k chunks: the boundary
                    # head's scores start with zero projection latency
                    force_due(qb + 1, 0)
                yi = 0
                for _ in attention_head(
                        qb, h, att_tiles,
                        lag=1 if (qb == 0 and h == 0) else 2):
                    yi += 1
                    if h == 0 and qb == 0:
                        # window 0's v slab is still streaming in: run the
                        # first contraction halves while the rest arrives
                        if yi == 1:
                            proj_v_phase(0, 0, 0)
                            proj_v_phase(0, 1, 0)
                        elif yi == 2:
                            proj_v_phase(0, 0, 1)
                            proj_v_phase(0, 1, 1)
                        elif yi == 3:
                            proj_v_phase(0, 2, 0)
                            proj_v_phase(0, 3, 0)
                            proj_v_phase(0, 2, 1)
                            proj_v_phase(0, 3, 1)
                    elif h == 0 and yi <= 2:
                        # v slab for this window's diagonal, 2 units per yield
                        for _ in range(2):
                            if proj_queue and proj_queue[0][2] == "v" \
                                    and proj_queue[0][3] == qb:
                                emit_proj_unit()
                    balance_filler(qb)
            # defer this window's Wo into the balance queue: it is the only
            # PE work with no deadline, so it belongs in the late ACT-bound
            # holes (atn/att bufs=4 make any emission order inversion-free)
            pqb, ptiles, pstore = qb, att_tiles, [None] * ndq

            def mk_tr(ptiles=ptiles, pstore=pstore, pqb=pqb):
                for cc in range(ndq):
                    wo_transpose_unit(ptiles, cc, pstore)
                if debug:
                    for qcl in range(4):
                        nc.sync.dma_start(
                            out=dbg_at[(pqb * 4 + qcl) * 128 :
                                       (pqb * 4 + qcl + 1) * 128, :],
                            in_=ptiles[qcl][:, :],
                        )

            if qb < nwin - 1:
                wo_queue.append(mk_tr)
                for i in range(4):
                    wo_queue.append(
                        lambda st=pstore, w=pqb, j=i: wo_matmul_unit(st, w, j))
            else:
                prev = (qb, att_tiles)
        while proj_queue:
            emit_proj_unit()
        while wo_queue:
            wo_queue.pop(0)()
        wo_full(*prev, last=True)

        if debug:
            for c in range(ndq):
                for w in range(nwin):
                    cs = slice(c * 128, (c + 1) * 128)
                    ws = slice(w * SQ, (w + 1) * SQ)
                    nc.sync.dma_start(out=dbg_q[cs, ws], in_=q_sb[c][w][:, :])
                    nc.sync.dma_start(out=dbg_k[cs, ws], in_=k_sb[c][w][:, :])
            for i in range(nsc):
                nc.sync.dma_start(
                    out=dbg_v[i * 128 : (i + 1) * 128, :], in_=v_sb[i][:, :]
                )

    if split_waits:
        split_excess_waits(nc)
    return nc


def make_crossmask():
    kk = np.arange(128)[:, None]
    qq = np.arange(128)[None, :]
    return np.where(kk <= qq, 0.0, -32768.0).astype(np.float16)


def _split8(a, axis):
    """fp8 (hi, lo) split of float32 `a`, stacked along a new `axis`."""
    hi = a.astype(NP_F8)
    lo = (a - hi.astype(np.float32)).astype(NP_F8)
    return np.stack([hi, lo], axis=axis)


def _x_fp8(xt):
    """[E, S] f32 -> [E, 2*S] fp8: per 512-col window, hi|lo interleaved."""
    xw = xt.reshape(E, S // SQ_H, SQ_H)
    return np.ascontiguousarray(_split8(xw, axis=2).reshape(E, -1))


def _w_fp8(wt):
    """[E, DQ] f32 (pre-scaled) -> [E, 2*DQ] fp8: hi|lo per e-row."""
    return np.ascontiguousarray(_split8(wt * WSCALE, axis=1).reshape(E, -1))


SQ_H = 512  # host-side copy of the kernel's q-window width


def classify_mask(mask):
    m = np.asarray(mask).reshape(S, S)
    if np.array_equal(m, np.tril(np.ones((S, S), bool))):
        return "causal"
    if m.all():
        return "dense"
    return "generic"


def prep_core_inputs(query, key, value, Wq, bq, Wk, bk, Wv, bv, Wo, bo, mask):
    """Shard + lay out host-side numpy inputs for the 8 cores."""
    kind = classify_mask(mask)
    maps = []
    xcache = {}
    for core in range(NCORES):
        b, gi = core // NGROUPS, core % NGROUPS
        gs = slice(gi * DQ, (gi + 1) * DQ)
        if b not in xcache:
            xcache[b] = {
                "xq8": _x_fp8(np.asarray(query[b], np.float32).T),
                "xk8": _x_fp8(np.asarray(key[b], np.float32).T),
                "xv8": _x_fp8(np.asarray(value[b], np.float32).T),
            }
        im = {
            **xcache[b],
            "wq8": _w_fp8(np.asarray(Wq, np.float32)[gs, :].T),
            "wk8": _w_fp8(np.asarray(Wk, np.float32)[gs, :].T),
            "wv8": _w_fp8(np.asarray(Wv, np.float32)[gs, :].T),
            "wo_t": np.ascontiguousarray(
                np.asarray(Wo)[:, gs].T.astype(np.float16)),
            "consts_f32": np.ascontiguousarray(np.concatenate([
                WSCALE * np.asarray(bq)[gs].astype(np.float32)
                .reshape(-1, 128).T,
                WSCALE * np.asarray(bk)[gs].astype(np.float32)
                .reshape(-1, 128).T,
                np.broadcast_to(
                    WSCALE * np.asarray(bv)[gs].astype(np.float32),
                    (128, DQ)),
            ], axis=1)),
            "consts_f16": np.ascontiguousarray(np.concatenate([
                np.eye(128, dtype=np.float16),
                32768.0 * np.eye(128, dtype=np.float16),
                make_crossmask(),
            ], axis=1)),
        }
        maps.append(im)
    return maps, kind


def make_runner(nc, n_cores=NCORES):
    """Build a reusable jitted SPMD executor for `nc` on cores 0..n_cores-1."""
    import jax
    from jax.experimental.shard_map import shard_map
    from jax.sharding import Mesh, PartitionSpec

    from concourse import bass2jax, mybir as _mybir

    bass2jax.install_neuronx_cc_hook()

    partition_name = (
        nc.partition_id_tensor.name if nc.partition_id_tensor else None
    )
    in_names, out_names, out_avals, zero_shapes = [], [], [], []
    for alloc in nc.m.functions[0].allocations:
        if not isinstance(alloc, _mybir.MemoryLocationSet):
            continue
        name = alloc.memorylocations[0].name
        if alloc.kind == "ExternalInput":
            if name != partition_name:
                in_names.append(name)
        elif alloc.kind == "ExternalOutput":
            out_names.append(name)
            shape = tuple(alloc.tensor_shape)
            dtype = _mybir.dt.np(alloc.dtype)
            out_avals.append(jax.core.ShapedArray(shape, dtype))
            zero_shapes.append((shape, dtype))
    n_params = len(in_names)
    all_in = list(in_names) + list(out_names)
    if partition_name is not None:
        all_in.append(partition_name)

    def _body(*args):
        operands = list(args)
        if partition_name is not None:
            operands.append(bass2jax.partition_id_tensor())
        outs = bass2jax._bass_exec_p.bind(
            *operands,
            out_avals=tuple(out_avals),
            in_names=tuple(all_in),
            out_names=tuple(out_names),
            lowering_input_output_aliases=(),
            sim_require_finite=True,
            sim_require_nnan=True,
            nc=nc,
        )
        return tuple(outs)

    devices = jax.devices()[:n_cores]
    assert len(devices) == n_cores
    mesh = Mesh(np.asarray(devices), ("core",))
    in_specs = (PartitionSpec("core"),) * (n_params + len(out_names))
    out_specs = (PartitionSpec("core"),) * len(out_names)
    sharded = jax.jit(
        shard_map(
            _body,
            mesh=mesh,
            in_specs=in_specs,
            out_specs=out_specs,
            check_rep=False,
        ),
        keep_unused=True,
    )
    zeros = [
        np.zeros((n_cores * sh[0], *sh[1:]), dt) for sh, dt in zero_shapes
    ]

    def concat_inputs(in_maps):
        return [
            np.concatenate(
                [np.asarray(in_maps[c][n]) for c in range(n_cores)], axis=0
            )
            for n in in_names
        ]

    def run(in_maps):
        out_arrs = sharded(*concat_inputs(in_maps), *zeros)
        return [
            {
                name: np.asarray(out_arrs[i]).reshape(
                    n_cores, *out_avals[i].shape
                )[c]
                for i, name in enumerate(out_names)
            }
            for c in range(n_cores)
        ]

    run.sharded = sharded
    run.concat_inputs = concat_inputs
    run.zeros = zeros
    run.out_names = out_names
    run.out_avals = out_avals
    return run


_CACHE = {}


def get_runner(kind="causal"):
    if kind not in _CACHE:
        nc = build_kernel(causal=(kind == "causal"))
        _CACHE[kind] = make_runner(nc)
    return _CACHE[kind]


def _numpy_reference(query, key, value, Wq, bq, Wk, bk, Wv, bv, Wo, bo, mask):
    q = (query @ Wq.T + bq).reshape(B, S, H, D).transpose(0, 2, 1, 3)
    k = (key @ Wk.T + bk).reshape(B, S, H, D).transpose(0, 2, 1, 3)
    v = (value @ Wv.T + bv).reshape(B, S, H, D).transpose(0, 2, 1, 3)
    sc = np.einsum("bhqd,bhkd->bhqk", q, k) / np.sqrt(D)
    sc = np.where(np.asarray(mask).reshape(1, 1, S, S), sc, -np.inf)
    sc -= sc.max(axis=-1, keepdims=True)
    p = np.exp(sc)
    p /= p.sum(axis=-1, keepdims=True)
    o = np.einsum("bhqk,bhkd->bhqd", p, v)
    o = o.transpose(0, 2, 1, 3).reshape(B, S, E)
    return o @ Wo.T + bo


def kernel(**inputs) -> np.ndarray:
    kind = classify_mask(inputs["mask"])
    if kind == "generic":
        fp = {k: np.asarray(v, np.float32) for k, v in inputs.items()
              if k != "mask"}
        return _numpy_reference(mask=inputs["mask"], **fp).astype(np.float32)
    in_maps, kind = prep_core_inputs(**inputs)
    run = get_runner(kind)
    results = run(in_maps)
    bo = np.asarray(inputs["bo"], dtype=np.float32)
    out = np.empty((B, S, E), dtype=np.float32)
    for b in range(B):
        acc = results[b * NGROUPS]["out"].astype(np.float32)
        for gi in range(1, NGROUPS):
            acc = acc + results[b * NGROUPS + gi]["out"].astype(np.float32)
        out[b] = acc + bo[None, :]
    return out

